# revision 4
# baseline (speedup 1.0000x reference)
"""BilateralGPT forward on 8 Trainium2 NeuronCores.

Sharding: core c = b*4 + s*2 + h  ->  (batch b, stream s in {main, analysis},
token-half h). Each core runs the full 6-layer trunk for its 512 tokens of its
(batch, stream), exchanging lateral activations (between stream pairs) and K/V
(between token-half pairs) via AllGather collectives. The vocab head runs on
all 8 cores: final activations are AllGathered within token-half pairs and each
core computes [1024 tokens, 16000 vocab] (vocab half selected by h).

The program is SPMD-uniform: all per-core asymmetry (token ranges, causal
masks, rope phases, which lateral-gather slot is "the other stream") is pushed
into per-core input data, never into instructions.
"""

import numpy as np
import ml_dtypes

import concourse.bacc as bacc
import concourse.tile as tile
import concourse.bass as bass
from concourse import mybir
from concourse.bass_utils import run_bass_kernel_spmd
from concourse.masks import make_identity

BF = ml_dtypes.bfloat16
F32 = np.float32
DT_BF = mybir.dt.bfloat16
DT_F32 = mybir.dt.float32

V, D, H, DH, NL, LAT, NLAYER = 32000, 768, 12, 64, 6, 128, 6
THETA, EPS, B, T = 10000.0, 1e-5, 2, 1024
R = 512            # tokens per core
TT = R // 128      # 4 token tiles per core
DTILES = D // 128  # 6 feature tiles
VH = V // 2        # vocab half per core
N_CORES = 8
KCH = 8            # real key chunks of 128 (the latent chunk is separate)

LAT_GROUPS = [[0, 2], [1, 3], [4, 6], [5, 7]]  # stream-pair exchange
KV_GROUPS = [[0, 1], [2, 3], [4, 5], [6, 7]]   # token-half-pair exchange


# ----------------------------------------------------------------- host prep

def _perm():
    """Deinterleave rope pairs within each head: new col h*64+i <- old h*64+2i,
    new col h*64+32+i <- old h*64+2i+1. Applied to Wq/Wk columns (and latent
    keys), it turns interleaved-pair rope into rotate-half rope while leaving
    attention scores unchanged."""
    p = np.zeros(D, dtype=np.int64)
    for h in range(H):
        for i in range(DH // 2):
            p[h * DH + i] = h * DH + 2 * i
            p[h * DH + DH // 2 + i] = h * DH + 2 * i + 1
    return p


def _fold_g(w, g):
    g = np.asarray(g, F32)
    if np.all(g == 1.0):
        return np.asarray(w, F32)
    return np.asarray(w, F32) * g[:, None]


def _check_b(b):
    if np.any(np.asarray(b, F32) != 0.0):
        raise NotImplementedError("nonzero LayerNorm bias not supported")


def _prep_inputs(idx, params):
    """Build per-core in_maps (host-side sharding + weight preprocessing)."""
    idx = np.asarray(idx)
    P = _perm()
    tok = np.asarray(params["tok_emb"], F32)
    pos = np.asarray(params["pos_emb"], F32)

    freqs = 1.0 / THETA ** (np.arange(0, DH, 2, dtype=F32) / DH)
    tpos = np.arange(T, dtype=F32)[:, None]
    cos_t = np.cos(tpos * freqs)  # [T, 32]
    sin_t = np.sin(tpos * freqs)

    blocks = params["blocks"]
    # per-stream weight tensors, shared by the 4 cores of each stream
    sw = [{} for _ in range(2)]
    for s in range(2):
        for l, blk in enumerate(blocks):
            at = blk["attn"]
            _check_b(blk["lat_pre"]["ln_b"][s])
            _check_b(blk["lat_post"]["ln_b"][s])
            _check_b(blk["ln1_b"][s])
            _check_b(blk["ln2_b"][s])
            w = sw[s]
            ln1g = np.asarray(blk["ln1_g"][s], F32)
            w[f"L{l}_wq"] = _fold_g(at["Wq"][s], ln1g)[:, P].astype(BF)
            w[f"L{l}_wk"] = _fold_g(at["Wk"][s], ln1g)[:, P].astype(BF)
            w[f"L{l}_wv"] = _fold_g(at["Wv"][s], ln1g).astype(BF)
            w[f"L{l}_wo"] = np.asarray(at["Wproj"][s], F32).astype(BF)
            lkf = (np.asarray(at["lk"][s], F32) @ np.asarray(at["Wlk"][s], F32))
            w[f"L{l}_lkT"] = np.ascontiguousarray(lkf[:, P].T).astype(BF)  # [768, 6]
            for tag, lat in (("pre", blk["lat_pre"]), ("post", blk["lat_post"])):
                projs = _fold_g(lat["proj"][s], np.asarray(lat["ln_g"][s], F32))
                w[f"L{l}_proj{tag}"] = projs.astype(BF)
                gate = np.asarray(lat["gate"][s], F32)  # [1536, 768]
                gx, gl = gate[:D], gate[D:]
                z = np.zeros_like(gl)
                # lhsT blocks are [x_self; recv0; recv1]; recv slots are in
                # group-rank order = [stream0, stream1]. Select the partner's
                # slot by zeroing the own-stream block of the weight.
                if s == 0:
                    ge = np.concatenate([gx, z, gl], axis=0)
                else:
                    ge = np.concatenate([gx, gl, z], axis=0)
                w[f"L{l}_gate{tag}"] = ge.astype(BF)  # [2304, 768]
            w[f"L{l}_w1"] = _fold_g(blk["ff_W1"][s], np.asarray(blk["ln2_g"][s], F32)).astype(BF)
            w[f"L{l}_w2"] = np.asarray(blk["ff_W2"][s], F32).astype(BF)
        _check_b(params["ln_f_b"][s])
        hw = _fold_g(params["head_w"][s], np.asarray(params["ln_f_g"][s], F32))
        sw[s]["_head"] = hw.astype(BF)  # [768, 32000]

    in_maps = []
    for c in range(N_CORES):
        b, s, h = c >> 2, (c >> 1) & 1, c & 1
        m = {}
        t0 = h * R
        x0 = tok[idx[b, t0:t0 + R]] + pos[t0:t0 + R]
        m["x0"] = np.ascontiguousarray(x0, dtype=F32)
        cq = np.concatenate([cos_t[t0:t0 + R] / 8.0, sin_t[t0:t0 + R] / 8.0], axis=1)
        ck = np.concatenate([cos_t[t0:t0 + R], sin_t[t0:t0 + R]], axis=1)
        m["csq"] = cq.astype(BF)  # [512, 64]
        m["csk"] = ck.astype(BF)
        # causal 0/1 masks per key chunk: mask[c][i][j] = 1 iff (512h + j) >= (128c + i)
        kg = (np.arange(KCH * 128) // 128)[:, None]  # chunk of key (unused)
        keys = np.arange(KCH * 128).reshape(KCH, 128, 1)
        qs = (t0 + np.arange(R)).reshape(1, 1, R)
        m["masks"] = (qs >= keys).astype(BF)  # [8, 128, 512]
        # bias column: -30000 for chunks entirely invisible to every q of this core
        bias = np.zeros((KCH, 128), dtype=F32)
        fully_future = keys[:, :, 0] > (t0 + R - 1)
        bias[fully_future] = -30000.0
        m["biascol"] = np.ascontiguousarray(bias.T)  # [128, 8]
        for name, arr in sw[s].items():
            if name == "_head":
                m["wh"] = np.ascontiguousarray(arr[:, h * VH:(h + 1) * VH])
            else:
                m[name] = arr
        in_maps.append(m)
    return in_maps


# ------------------------------------------------------------ program builder

def _build(nlayer=NLAYER, dbg=False):
    nc = bacc.Bacc("TRN2", target_bir_lowering=False, debug=False,
                   num_devices=N_CORES)

    inp = {}
    inp["x0"] = nc.dram_tensor("x0", [R, D], DT_F32, kind="ExternalInput").ap()
    inp["csq"] = nc.dram_tensor("csq", [R, 64], DT_BF, kind="ExternalInput").ap()
    inp["csk"] = nc.dram_tensor("csk", [R, 64], DT_BF, kind="ExternalInput").ap()
    inp["masks"] = nc.dram_tensor("masks", [KCH, 128, R], DT_BF, kind="ExternalInput").ap()
    inp["biascol"] = nc.dram_tensor("biascol", [128, KCH], DT_F32, kind="ExternalInput").ap()
    for l in range(nlayer):
        for nm, shp in (("projpre", [D, D]), ("gatepre", [3 * D, D]),
                        ("wq", [D, D]), ("wk", [D, D]), ("wv", [D, D]),
                        ("lkT", [D, NL]), ("wo", [D, D]),
                        ("projpost", [D, D]), ("gatepost", [3 * D, D]),
                        ("w1", [D, 4 * D]), ("w2", [4 * D, D])):
            inp[f"L{l}_{nm}"] = nc.dram_tensor(f"L{l}_{nm}", shp, DT_BF,
                                               kind="ExternalInput").ap()
    inp["wh"] = nc.dram_tensor("wh", [D, VH], DT_BF, kind="ExternalInput").ap()
    out_log = nc.dram_tensor("logits", [2 * R, VH], DT_F32, kind="ExternalOutput").ap()
    out_dbg = None
    if dbg:
        out_dbg = nc.dram_tensor("dbgx", [R, D], DT_F32, kind="ExternalOutput").ap()

    with tile.TileContext(nc) as tc:
        _body(tc, inp, out_log, out_dbg, nlayer)
    nc.compile()
    return nc


def _body(tc, inp, out_log, out_dbg, nlayer):
    from contextlib import ExitStack
    nc = tc.nc
    AL = mybir.AluOpType
    AF = mybir.ActivationFunctionType

    es = ExitStack()
    pc = es.enter_context(tc.tile_pool(name="const", bufs=1))
    pst = es.enter_context(tc.tile_pool(name="small", bufs=4))
    wp = es.enter_context(tc.tile_pool(name="weights", bufs=3))  # 18K slots
    ps384 = es.enter_context(tc.tile_pool(name="ps384", bufs=4, space="PSUM"))
    ps512 = es.enter_context(tc.tile_pool(name="ps512", bufs=2, space="PSUM"))
    psav = es.enter_context(tc.tile_pool(name="psav", bufs=2, space="PSUM"))
    dram = es.enter_context(tc.tile_pool(name="dram", bufs=2, space="DRAM"))

    idf = pc.tile([128, 128], DT_F32)
    make_identity(nc, idf)
    idb = pc.tile([128, 128], DT_BF)
    nc.vector.tensor_copy(out=idb, in_=idf)
    epst = pc.tile([128, 1], DT_F32)
    nc.vector.memset(epst, EPS)
    masks = pc.tile([128, KCH, R], DT_BF)
    nc.sync.dma_start(out=masks, in_=inp["masks"].rearrange("c p q -> p c q"))
    biascol = pc.tile([128, KCH], DT_F32)
    nc.sync.dma_start(out=biascol, in_=inp["biascol"])
    csq = pc.tile([128, TT, 64], DT_BF)
    nc.sync.dma_start(out=csq, in_=inp["csq"].rearrange("(t p) i -> p t i", p=128))
    csk = pc.tile([128, TT, 64], DT_BF)
    nc.sync.dma_start(out=csk, in_=inp["csk"].rearrange("(t p) i -> p t i", p=128))

    x = pc.tile([128, TT, D], DT_F32)
    nc.sync.dma_start(out=x, in_=inp["x0"].rearrange("(t p) d -> p t d", p=128))

    def w_tile(name, ktiles, ncols, tag="wbig"):
        """DMA a [ktiles*128, ncols] bf16 weight into SBUF [128, ktiles, ncols]."""
        wt = wp.tile([128, ktiles, ncols], DT_BF, tag=tag)
        nc.sync.dma_start(out=wt, in_=name.rearrange("(k p) n -> p k n", p=128))
        return wt

    def layernorm(pool, tag):
        """rm layernorm of x -> bf16 [128, TT, D] (gain folded into weights)."""
        ln = pool.tile([128, TT, D], DT_BF, tag=tag)
        for t in range(TT):
            stats = pst.tile([128, 2, 6], DT_F32, tag="stats")
            for g2 in range(2):
                nc.vector.bn_stats(out=stats[:, g2, :], in_=x[:, t, g2 * 384:(g2 + 1) * 384])
            mv = pst.tile([128, 2], DT_F32, tag="mv")
            nc.vector.bn_aggr(out=mv, in_=stats)
            std = pst.tile([128, 1], DT_F32, tag="std")
            nc.scalar.activation(out=std, in_=mv[:, 1:2], func=AF.Sqrt, bias=epst)
            istd = pst.tile([128, 1], DT_F32, tag="istd")
            nc.vector.reciprocal(out=istd, in_=std)
            nc.vector.tensor_scalar(out=ln[:, t, :], in0=x[:, t, :],
                                    scalar1=mv[:, 0:1], scalar2=istd,
                                    op0=AL.subtract, op1=AL.mult)
        return ln

    def rm_to_fm(rm, pool, tag, src_f32=False):
        """Transpose [128, TT, D] rm -> [128, DTILES, R] fm bf16 via PE."""
        fm = pool.tile([128, DTILES, R], DT_BF, tag=tag)
        for d in range(DTILES):
            pt = ps512.tile([128, R], DT_F32 if src_f32 else DT_BF, tag="s512")
            for t in range(TT):
                nc.tensor.transpose(pt[:, t * 128:(t + 1) * 128],
                                    rm[:, t, d * 128:(d + 1) * 128],
                                    idf if src_f32 else idb)
            nc.scalar.copy(out=fm[:, d, :], in_=pt)
        return fm

    def lateral(l, tag):
        with tc.tile_pool(name=f"l{l}lat{tag}", bufs=1) as pl:
            wproj = w_tile(inp[f"L{l}_proj{tag}"], DTILES, D)
            # two halves of the [2304, 768] extended gate weight
            wg0 = w_tile(inp[f"L{l}_gate{tag}"][0:9 * 128, :], 9, D)
            wg1 = w_tile(inp[f"L{l}_gate{tag}"][9 * 128:18 * 128, :], 9, D)
            lnl = layernorm(pl, "ln_rm")
            ln_fm = rm_to_fm(lnl, pl, "ln_fm")
            x_fm = rm_to_fm(x, pl, "x_fm", src_f32=True)
            # l_self = LN(x) @ proj, emitted feature-major
            lself = pl.tile([128, DTILES, R], DT_BF, tag="lself")
            for n in range(DTILES):
                pt = ps512.tile([128, R], DT_F32, tag="s512")
                for k in range(DTILES):
                    nc.tensor.matmul(pt, lhsT=wproj[:, k, n * 128:(n + 1) * 128],
                                     rhs=ln_fm[:, k, :],
                                     start=(k == 0), stop=(k == DTILES - 1))
                nc.scalar.copy(out=lself[:, n, :], in_=pt)
            ls = dram.tile([128, DTILES * R], DT_BF, tag="ls")
            nc.sync.dma_start(out=ls, in_=lself)
            lr = dram.tile([2, 128, DTILES * R], DT_BF, tag="lr")
            nc.gpsimd.collective_compute(
                "AllGather", AL.bypass, replica_groups=LAT_GROUPS,
                ins=[ls[:].opt()], outs=[lr[:].opt()])
            l01 = pl.tile([128, 2 * DTILES, R], DT_BF, tag="l01")
            for r2 in range(2):
                nc.sync.dma_start(
                    out=l01[:, r2 * DTILES:(r2 + 1) * DTILES, :],
                    in_=lr[r2].rearrange("p (k n) -> p k n", k=DTILES))
            # gate = sigmoid([x; recv0; recv1] @ [gx; A; B]); own-stream slot zeroed
            for t in range(TT):
                g = pl.tile([128, D], DT_BF, tag="g")
                for half in range(2):
                    pt = ps384.tile([128, 384], DT_F32, tag="p384")
                    for k in range(18):
                        if k < DTILES:
                            lhs = x_fm[:, k, t * 128:(t + 1) * 128]
                        else:
                            lhs = l01[:, k - DTILES, t * 128:(t + 1) * 128]
                        wg = wg0 if k < 9 else wg1
                        nc.tensor.matmul(pt, lhsT=lhs,
                                         rhs=wg[:, k % 9, half * 384:(half + 1) * 384],
                                         start=(k == 0), stop=(k == 17))
                    nc.scalar.activation(out=g[:, half * 384:(half + 1) * 384],
                                         in_=pt, func=AF.Sigmoid)
                nc.vector.tensor_tensor(out=x[:, t, :], in0=x[:, t, :], in1=g,
                                        op=AL.mult)

    def rope(psums, cs, dst, t):
        """Apply rotate-half rope to one token tile. psums = [psum heads 0-5,
        psum heads 6-11] each [128, 384] fp32; dst bf16 [128, TT, D]."""
        for ph in range(2):
            raw = pst.tile([128, 384], DT_BF, tag="rope_raw")
            nc.scalar.copy(out=raw, in_=psums[ph])
            rv = raw.rearrange("p (h two i) -> p h two i", h=6, two=2)
            dv = dst[:, t, ph * 384:(ph + 1) * 384].rearrange(
                "p (h two i) -> p h two i", h=6, two=2)
            c_ap = cs[:, t, 0:32]
            s_ap = cs[:, t, 32:64]
            cb = bass.AP(tensor=c_ap.tensor, offset=c_ap.offset,
                         ap=[c_ap.ap[0], [0, 6], c_ap.ap[1]])
            sb = bass.AP(tensor=s_ap.tensor, offset=s_ap.offset,
                         ap=[s_ap.ap[0], [0, 6], s_ap.ap[1]])
            t1 = pst.tile([128, 6, 32], DT_BF, tag="ropet1")
            t2 = pst.tile([128, 6, 32], DT_BF, tag="ropet2")
            q1, q2 = rv[:, :, 0, :], rv[:, :, 1, :]
            nc.vector.tensor_tensor(out=t1, in0=q1, in1=cb, op=AL.mult)
            nc.vector.tensor_tensor(out=t2, in0=q2, in1=sb, op=AL.mult)
            nc.vector.tensor_tensor(out=dv[:, :, 0, :], in0=t1, in1=t2, op=AL.subtract)
            nc.vector.tensor_tensor(out=t1, in0=q1, in1=sb, op=AL.mult)
            nc.vector.tensor_tensor(out=t2, in0=q2, in1=cb, op=AL.mult)
            nc.vector.tensor_tensor(out=dv[:, :, 1, :], in0=t1, in1=t2, op=AL.add)

    def attention(l):
        with tc.tile_pool(name=f"l{l}att", bufs=1) as pa:
            wlk = w_tile(inp[f"L{l}_lkT"], DTILES, NL, tag="wlk")
            wq = w_tile(inp[f"L{l}_wq"], DTILES, D)
            wk = w_tile(inp[f"L{l}_wk"], DTILES, D)
            wv = w_tile(inp[f"L{l}_wv"], DTILES, D)
            ln1 = layernorm(pa, "ln_rm")
            ln_fm = rm_to_fm(ln1, pa, "ln_fm")

            with tc.tile_pool(name=f"l{l}qkv", bufs=1) as pq:
                q_rm = pq.tile([128, TT, D], DT_BF, tag="q_rm")
                k_rm = pq.tile([128, TT, D], DT_BF, tag="k_rm")
                v_loc = pa.tile([128, TT, H * 65], DT_BF, tag="v_loc")
                for t in range(TT):
                    for w_, cs_, dst in ((wq, csq, q_rm), (wk, csk, k_rm)):
                        pts = []
                        for half in range(2):
                            pt = ps384.tile([128, 384], DT_F32, tag="p384")
                            for k in range(DTILES):
                                nc.tensor.matmul(
                                    pt, lhsT=ln_fm[:, k, t * 128:(t + 1) * 128],
                                    rhs=w_[:, k, half * 384:(half + 1) * 384],
                                    start=(k == 0), stop=(k == DTILES - 1))
                            pts.append(pt)
                        rope(pts, cs_, dst, t)
                    for half in range(2):
                        pt = ps384.tile([128, 384], DT_F32, tag="p384")
                        for k in range(DTILES):
                            nc.tensor.matmul(
                                pt, lhsT=ln_fm[:, k, t * 128:(t + 1) * 128],
                                rhs=wv[:, k, half * 384:(half + 1) * 384],
                                start=(k == 0), stop=(k == DTILES - 1))
                        # scatter the 6 heads of this half into v_aug layout
                        pv = pt.rearrange("p (h d) -> p h d", h=6)
                        vv = v_loc[:, t, half * 6 * 65:(half + 1) * 6 * 65]
                        vv = vv.rearrange("p (h d) -> p h d", h=6)
                        nc.vector.tensor_copy(out=vv[:, :, 0:64], in_=pv)
                    ones_cols = v_loc[:, t, :].rearrange("p (h d) -> p h d", h=H)
                    nc.vector.memset(ones_cols[:, :, 64:65], 1.0)
                q_fm = pa.tile([128, DTILES, R], DT_BF, tag="q_fm")
                k_fm = pa.tile([128, DTILES, R], DT_BF, tag="k_fm")
                for d in range(DTILES):
                    for src, dstt in ((q_rm, q_fm), (k_rm, k_fm)):
                        ptt = ps512.tile([128, R], DT_BF, tag="s512")
                        for t in range(TT):
                            nc.tensor.transpose(ptt[:, t * 128:(t + 1) * 128],
                                                src[:, t, d * 128:(d + 1) * 128], idb)
                        nc.scalar.copy(out=dstt[:, d, :], in_=ptt)

            # k/v exchange within token-half pair (global key order = rank order)
            KB = DTILES * R          # 3072 bf16 per partition for k
            VB = TT * H * 65         # 3120 for v
            kvs = dram.tile([128, KB + VB], DT_BF, tag="kvs")
            nc.sync.dma_start(out=kvs[:, 0:KB].rearrange("p (k n) -> p k n", k=DTILES),
                              in_=k_fm)
            nc.sync.dma_start(out=kvs[:, KB:].rearrange("p (t n) -> p t n", t=TT),
                              in_=v_loc)
            kvr = dram.tile([2, 128, KB + VB], DT_BF, tag="kvr")
            nc.gpsimd.collective_compute(
                "AllGather", AL.bypass, replica_groups=KV_GROUPS,
                ins=[kvs[:].opt()], outs=[kvr[:].opt()])
            k_all = pa.tile([128, DTILES, T + NL], DT_BF, tag="k_all")
            v_all = pa.tile([128, KCH + 1, H * 65], DT_BF, tag="v_all")
            for r2 in range(2):
                nc.sync.dma_start(
                    out=k_all[:, :, r2 * R:(r2 + 1) * R],
                    in_=kvr[r2, :, 0:KB].rearrange("p (k n) -> p k n", k=DTILES))
                nc.sync.dma_start(
                    out=v_all[:, r2 * TT:(r2 + 1) * TT, :],
                    in_=kvr[r2, :, KB:].rearrange("p (t n) -> p t n", t=TT))
            nc.sync.dma_start(out=k_all[:, :, T:T + NL], in_=wlk)
            nc.vector.memset(v_all[0:NL, KCH, :], 0.0)
            vlat = v_all[0:NL, KCH, :].rearrange("p (h d) -> p h d", h=H)
            nc.vector.memset(vlat[:, :, 64:65], 1.0)

            att_rm = pa.tile([128, TT, D], DT_BF, tag="att_rm")
            for h in range(H):
                po = 64 * (h % 2)
                ft = h // 2
                exp_sb = pa.tile([128, KCH, R], DT_BF, tag="exp_sb")
                exp_lat = pa.tile([NL, R], DT_BF, tag="exp_lat")
                for ch in range(KCH):
                    pt = ps512.tile([128, R], DT_F32, tag="s512")
                    nc.tensor.matmul(
                        pt, lhsT=k_all[po:po + 64, ft, ch * 128:(ch + 1) * 128],
                        rhs=q_fm[po:po + 64, ft, :], start=True, stop=True)
                    nc.scalar.activation(out=exp_sb[:, ch, :], in_=pt, func=AF.Exp,
                                         bias=biascol[:, ch:ch + 1])
                    ext = 128 * (ch % 4 + 1)
                    nc.vector.tensor_tensor(out=exp_sb[:, ch, 0:ext],
                                            in0=exp_sb[:, ch, 0:ext],
                                            in1=masks[:, ch, 0:ext], op=AL.mult)
                ptl = ps512.tile([NL, R], DT_F32, tag="s512")
                nc.tensor.matmul(ptl, lhsT=wlk[po:po + 64, ft, :],
                                 rhs=q_fm[po:po + 64, ft, :], start=True, stop=True)
                nc.scalar.activation(out=exp_lat, in_=ptl, func=AF.Exp)
                for t in range(TT):
                    pav = psav.tile([128, 65], DT_F32, tag="pav")
                    for ch in range(KCH):
                        nc.tensor.matmul(pav,
                                         lhsT=exp_sb[:, ch, t * 128:(t + 1) * 128],
                                         rhs=v_all[:, ch, h * 65:(h + 1) * 65],
                                         start=(ch == 0), stop=False)
                    nc.tensor.matmul(pav, lhsT=exp_lat[:, t * 128:(t + 1) * 128],
                                     rhs=v_all[0:NL, KCH, h * 65:(h + 1) * 65],
                                     start=False, stop=True)
                    rec = pst.tile([128, 1], DT_F32, tag="rec")
                    nc.vector.reciprocal(out=rec, in_=pav[:, 64:65])
                    nc.vector.tensor_scalar(out=att_rm[:, t, h * 64:(h + 1) * 64],
                                            in0=pav[:, 0:64], scalar1=rec,
                                            scalar2=None, op0=AL.mult)

            att_fm = rm_to_fm(att_rm, pa, "att_fm")
            wo = w_tile(inp[f"L{l}_wo"], DTILES, D)
            for t in range(TT):
                for half in range(2):
                    pt = ps384.tile([128, 384], DT_F32, tag="p384")
                    for k in range(DTILES):
                        nc.tensor.matmul(pt, lhsT=att_fm[:, k, t * 128:(t + 1) * 128],
                                         rhs=wo[:, k, half * 384:(half + 1) * 384],
                                         start=(k == 0), stop=(k == DTILES - 1))
                    nc.vector.tensor_tensor(out=x[:, t, half * 384:(half + 1) * 384],
                                            in0=pt,
                                            in1=x[:, t, half * 384:(half + 1) * 384],
                                            op=AL.add)

    def ffn(l):
        with tc.tile_pool(name=f"l{l}ffn", bufs=1) as pf:
            ln2 = layernorm(pf, "ln_rm")
            ln_fm = rm_to_fm(ln2, pf, "ln_fm")
            mid = pf.tile([128, 24, R], DT_BF, tag="mid")
            for half in range(2):
                w1h = w_tile(inp[f"L{l}_w1"][:, half * 1536:(half + 1) * 1536], DTILES, 1536)
                for ml in range(12):
                    pt = ps512.tile([128, R], DT_F32, tag="s512")
                    for k in range(DTILES):
                        nc.tensor.matmul(pt, lhsT=w1h[:, k, ml * 128:(ml + 1) * 128],
                                         rhs=ln_fm[:, k, :],
                                         start=(k == 0), stop=(k == DTILES - 1))
                    nc.scalar.activation(out=mid[:, half * 12 + ml, :], in_=pt,
                                         func=AF.Gelu)
            w2h0 = w_tile(inp[f"L{l}_w2"][0:1536, :], 12, D)
            w2h1 = w_tile(inp[f"L{l}_w2"][1536:3072, :], 12, D)
            for t in range(TT):
                for half in range(2):
                    pt = ps384.tile([128, 384], DT_F32, tag="p384")
                    for m in range(24):
                        w2h = w2h0 if m < 12 else w2h1
                        nc.tensor.matmul(pt, lhsT=mid[:, m, t * 128:(t + 1) * 128],
                                         rhs=w2h[:, m % 12, half * 384:(half + 1) * 384],
                                         start=(m == 0), stop=(m == 23))
                    nc.vector.tensor_tensor(out=x[:, t, half * 384:(half + 1) * 384],
                                            in0=pt,
                                            in1=x[:, t, half * 384:(half + 1) * 384],
                                            op=AL.add)

    for l in range(nlayer):
        lateral(l, "pre")
        attention(l)
        lateral(l, "post")
        ffn(l)

    if out_dbg is not None:
        nc.sync.dma_start(out=out_dbg.rearrange("(t p) d -> p t d", p=128), in_=x)

    # final layernorm + exchange + vocab head
    with tc.tile_pool(name="head", bufs=1) as ph:
        lnf = layernorm(ph, "ln_rm")
        lnf_fm = rm_to_fm(lnf, ph, "ln_fm")
        fs = dram.tile([128, DTILES * R], DT_BF, tag="fs")
        nc.sync.dma_start(out=fs, in_=lnf_fm)
        fr = dram.tile([2, 128, DTILES * R], DT_BF, tag="fr")
        nc.gpsimd.collective_compute(
            "AllGather", AL.bypass, replica_groups=KV_GROUPS,
            ins=[fs[:].opt()], outs=[fr[:].opt()])
        lnf_all = ph.tile([128, DTILES, 2 * R], DT_BF, tag="lnf_all")
        for r2 in range(2):
            nc.sync.dma_start(
                out=lnf_all[:, :, r2 * R:(r2 + 1) * R],
                in_=fr[r2].rearrange("p (k n) -> p k n", k=DTILES))
        NV = 1000
        for vc in range(VH // NV):
            whs = wp.tile([128, DTILES, NV], DT_BF, tag="wbig")
            nc.sync.dma_start(out=whs,
                              in_=inp["wh"][:, vc * NV:(vc + 1) * NV].rearrange(
                                  "(k p) n -> p k n", p=128))
            for t8 in range(2 * TT):
                for half in range(2):
                    pt = ps512.tile([128, NV // 2], DT_F32, tag="s512")
                    for k in range(DTILES):
                        nc.tensor.matmul(
                            pt, lhsT=lnf_all[:, k, t8 * 128:(t8 + 1) * 128],
                            rhs=whs[:, k, half * 500:(half + 1) * 500],
                            start=(k == 0), stop=(k == DTILES - 1))
                    ot = ph.tile([128, NV // 2], DT_F32, tag="ot")
                    nc.scalar.copy(out=ot, in_=pt)
                    nc.sync.dma_start(
                        out=out_log[t8 * 128:(t8 + 1) * 128,
                                    vc * NV + half * 500:vc * NV + (half + 1) * 500],
                        in_=ot)
    es.close()


# ----------------------------------------------------------------- interface

_PROG = {}


def _get_prog(nlayer=NLAYER, dbg=False):
    key = (nlayer, dbg)
    if key not in _PROG:
        _PROG[key] = _build(nlayer, dbg)
    return _PROG[key]


LAST_RESULT = None


def kernel(idx, params, _nlayer=NLAYER, _dbg=False):
    global LAST_RESULT
    idx = np.asarray(idx)
    in_maps = _prep_inputs(idx, params)
    nc = _get_prog(_nlayer, _dbg)
    res = run_bass_kernel_spmd(nc, in_maps, core_ids=list(range(N_CORES)))
    LAST_RESULT = res
    lm = np.empty((B, T, V), dtype=F32)
    la = np.empty((B, T, V), dtype=F32)
    for c in range(N_CORES):
        b, s, h = c >> 2, (c >> 1) & 1, c & 1
        dst = lm if s == 0 else la
        dst[b, :, h * VH:(h + 1) * VH] = res.results[c]["logits"]
    if _dbg:
        dbg = [res.results[c]["dbgx"] for c in range(N_CORES)]
        return (lm, la), dbg
    return (lm, la)


# revision 6
# speedup vs baseline: 1.1665x; 1.1665x over previous
"""BilateralGPT forward on 8 Trainium2 NeuronCores.

Sharding: core c = b*4 + s*2 + h  ->  (batch b, stream s in {main, analysis},
token-half h). Each core runs the full 6-layer trunk for its 512 tokens of its
(batch, stream), exchanging lateral activations (between stream pairs) and K/V
(between token-half pairs) via AllGather collectives. The vocab head runs on
all 8 cores: final activations are AllGathered within token-half pairs and each
core computes [1024 tokens, 16000 vocab] (vocab half selected by h).

The program is SPMD-uniform: all per-core asymmetry (token ranges, causal
masks, rope phases, which lateral-gather slot is "the other stream") is pushed
into per-core input data, never into instructions.
"""

import numpy as np
import ml_dtypes

import concourse.bacc as bacc
import concourse.tile as tile
import concourse.bass as bass
from concourse import mybir
from concourse.bass_utils import run_bass_kernel_spmd
from concourse.masks import make_identity

BF = ml_dtypes.bfloat16
F32 = np.float32
DT_BF = mybir.dt.bfloat16
DT_F32 = mybir.dt.float32

V, D, H, DH, NL, LAT, NLAYER = 32000, 768, 12, 64, 6, 128, 6
THETA, EPS, B, T = 10000.0, 1e-5, 2, 1024
R = 512            # tokens per core
TT = R // 128      # 4 token tiles per core
DTILES = D // 128  # 6 feature tiles
VH = V // 2        # vocab half per core
N_CORES = 8
KCH = 8            # real key chunks of 128 (the latent chunk is separate)

LAT_GROUPS = [[0, 2], [1, 3], [4, 6], [5, 7]]  # stream-pair exchange
KV_GROUPS = [[0, 1], [2, 3], [4, 5], [6, 7]]   # token-half-pair exchange


# ----------------------------------------------------------------- host prep

def _perm():
    """Deinterleave rope pairs within each head: new col h*64+i <- old h*64+2i,
    new col h*64+32+i <- old h*64+2i+1. Applied to Wq/Wk columns (and latent
    keys), it turns interleaved-pair rope into rotate-half rope while leaving
    attention scores unchanged."""
    p = np.zeros(D, dtype=np.int64)
    for h in range(H):
        for i in range(DH // 2):
            p[h * DH + i] = h * DH + 2 * i
            p[h * DH + DH // 2 + i] = h * DH + 2 * i + 1
    return p


def _fold_g(w, g):
    g = np.asarray(g, F32)
    if np.all(g == 1.0):
        return np.asarray(w, F32)
    return np.asarray(w, F32) * g[:, None]


def _check_b(b):
    if np.any(np.asarray(b, F32) != 0.0):
        raise NotImplementedError("nonzero LayerNorm bias not supported")


def _rl(w, ktiles):
    """[ktiles*128, N] -> [128, ktiles, N] so each SBUF partition's DMA data
    is one (or few) contiguous runs instead of 128-strided gathers."""
    a = np.asarray(w)
    return np.ascontiguousarray(a.reshape(ktiles, 128, a.shape[-1]).transpose(1, 0, 2))


def _prep_inputs(idx, params):
    """Build per-core in_maps (host-side sharding + weight preprocessing)."""
    idx = np.asarray(idx)
    P = _perm()
    tok = np.asarray(params["tok_emb"], F32)
    pos = np.asarray(params["pos_emb"], F32)

    freqs = 1.0 / THETA ** (np.arange(0, DH, 2, dtype=F32) / DH)
    tpos = np.arange(T, dtype=F32)[:, None]
    cos_t = np.cos(tpos * freqs)  # [T, 32]
    sin_t = np.sin(tpos * freqs)

    blocks = params["blocks"]
    # per-stream weight tensors, shared by the 4 cores of each stream
    sw = [{} for _ in range(2)]
    for s in range(2):
        for l, blk in enumerate(blocks):
            at = blk["attn"]
            _check_b(blk["lat_pre"]["ln_b"][s])
            _check_b(blk["lat_post"]["ln_b"][s])
            _check_b(blk["ln1_b"][s])
            _check_b(blk["ln2_b"][s])
            w = sw[s]
            ln1g = np.asarray(blk["ln1_g"][s], F32)
            w[f"L{l}_wq"] = _rl(_fold_g(at["Wq"][s], ln1g)[:, P].astype(BF), DTILES)
            w[f"L{l}_wk"] = _rl(_fold_g(at["Wk"][s], ln1g)[:, P].astype(BF), DTILES)
            w[f"L{l}_wv"] = _rl(_fold_g(at["Wv"][s], ln1g).astype(BF), DTILES)
            w[f"L{l}_wo"] = _rl(np.asarray(at["Wproj"][s], F32).astype(BF), DTILES)
            lkf = (np.asarray(at["lk"][s], F32) @ np.asarray(at["Wlk"][s], F32))
            w[f"L{l}_lkT"] = _rl(np.ascontiguousarray(lkf[:, P].T).astype(BF), DTILES)
            for tag, lat in (("pre", blk["lat_pre"]), ("post", blk["lat_post"])):
                projs = _fold_g(lat["proj"][s], np.asarray(lat["ln_g"][s], F32))
                w[f"L{l}_proj{tag}"] = _rl(projs.astype(BF), DTILES)
                gate = np.asarray(lat["gate"][s], F32)  # [1536, 768]
                gx, gl = gate[:D], gate[D:]
                z = np.zeros_like(gl)
                # lhsT blocks are [x_self; recv0; recv1]; recv slots are in
                # group-rank order = [stream0, stream1]. Select the partner's
                # slot by zeroing the own-stream block of the weight.
                if s == 0:
                    ge = np.concatenate([gx, z, gl], axis=0)
                else:
                    ge = np.concatenate([gx, gl, z], axis=0)
                w[f"L{l}_gate{tag}"] = _rl(ge.astype(BF), 18)  # [128, 18, 768]
            w[f"L{l}_w1"] = _rl(_fold_g(blk["ff_W1"][s], np.asarray(blk["ln2_g"][s], F32)).astype(BF), DTILES)
            w[f"L{l}_w2"] = _rl(np.asarray(blk["ff_W2"][s], F32).astype(BF), 24)
        _check_b(params["ln_f_b"][s])
        hw = _fold_g(params["head_w"][s], np.asarray(params["ln_f_g"][s], F32))
        sw[s]["_head"] = _rl(hw.astype(BF), DTILES)  # [128, 6, 32000]

    in_maps = []
    for c in range(N_CORES):
        b, s, h = c >> 2, (c >> 1) & 1, c & 1
        m = {}
        t0 = h * R
        x0 = tok[idx[b, t0:t0 + R]] + pos[t0:t0 + R]
        m["x0"] = _rl(np.ascontiguousarray(x0, dtype=F32), TT)
        cq = np.concatenate([cos_t[t0:t0 + R] / 8.0, sin_t[t0:t0 + R] / 8.0], axis=1)
        ck = np.concatenate([cos_t[t0:t0 + R], sin_t[t0:t0 + R]], axis=1)
        m["csq"] = _rl(cq.astype(BF), TT)  # [128, 4, 64]
        m["csk"] = _rl(ck.astype(BF), TT)
        # causal 0/1 masks per key chunk: mask[c][i][j] = 1 iff (512h + j) >= (128c + i)
        kg = (np.arange(KCH * 128) // 128)[:, None]  # chunk of key (unused)
        keys = np.arange(KCH * 128).reshape(KCH, 128, 1)
        qs = (t0 + np.arange(R)).reshape(1, 1, R)
        mk = (qs >= keys).astype(BF)  # [8, 128, 512]
        m["masks"] = np.ascontiguousarray(mk.transpose(1, 0, 2))  # [128, 8, 512]
        # bias column: -30000 for chunks entirely invisible to every q of this core
        bias = np.zeros((KCH, 128), dtype=F32)
        fully_future = keys[:, :, 0] > (t0 + R - 1)
        bias[fully_future] = -30000.0
        m["biascol"] = np.ascontiguousarray(bias.T)  # [128, 8]
        for name, arr in sw[s].items():
            if name == "_head":
                m["wh"] = np.ascontiguousarray(arr[:, :, h * VH:(h + 1) * VH])
            else:
                m[name] = arr
        in_maps.append(m)
    return in_maps


# ------------------------------------------------------------ program builder

def _build(nlayer=NLAYER, dbg=False):
    nc = bacc.Bacc("TRN2", target_bir_lowering=False, debug=False,
                   num_devices=N_CORES)

    inp = {}
    inp["x0"] = nc.dram_tensor("x0", [128, TT, D], DT_F32, kind="ExternalInput").ap()
    inp["csq"] = nc.dram_tensor("csq", [128, TT, 64], DT_BF, kind="ExternalInput").ap()
    inp["csk"] = nc.dram_tensor("csk", [128, TT, 64], DT_BF, kind="ExternalInput").ap()
    inp["masks"] = nc.dram_tensor("masks", [128, KCH, R], DT_BF, kind="ExternalInput").ap()
    inp["biascol"] = nc.dram_tensor("biascol", [128, KCH], DT_F32, kind="ExternalInput").ap()
    for l in range(nlayer):
        for nm, shp in (("projpre", [128, DTILES, D]), ("gatepre", [128, 18, D]),
                        ("wq", [128, DTILES, D]), ("wk", [128, DTILES, D]),
                        ("wv", [128, DTILES, D]), ("lkT", [128, DTILES, NL]),
                        ("wo", [128, DTILES, D]),
                        ("projpost", [128, DTILES, D]), ("gatepost", [128, 18, D]),
                        ("w1", [128, DTILES, 4 * D]), ("w2", [128, 24, D])):
            inp[f"L{l}_{nm}"] = nc.dram_tensor(f"L{l}_{nm}", shp, DT_BF,
                                               kind="ExternalInput").ap()
    inp["wh"] = nc.dram_tensor("wh", [128, DTILES, VH], DT_BF, kind="ExternalInput").ap()
    out_log = nc.dram_tensor("logits", [2 * R, VH], DT_F32, kind="ExternalOutput").ap()
    out_dbg = None
    if dbg:
        out_dbg = nc.dram_tensor("dbgx", [R, D], DT_F32, kind="ExternalOutput").ap()

    with tile.TileContext(nc) as tc:
        _body(tc, inp, out_log, out_dbg, nlayer)
    nc.compile()
    return nc


def _body(tc, inp, out_log, out_dbg, nlayer):
    from contextlib import ExitStack
    nc = tc.nc
    AL = mybir.AluOpType
    AF = mybir.ActivationFunctionType

    es = ExitStack()
    pc = es.enter_context(tc.tile_pool(name="const", bufs=1))
    pst = es.enter_context(tc.tile_pool(name="small", bufs=4))
    wp = es.enter_context(tc.tile_pool(name="weights", bufs=3))  # 18K slots
    ps384 = es.enter_context(tc.tile_pool(name="ps384", bufs=2, space="PSUM"))
    ps512 = es.enter_context(tc.tile_pool(name="ps512", bufs=2, space="PSUM"))
    psav = es.enter_context(tc.tile_pool(name="psav", bufs=2, space="PSUM"))
    dram = es.enter_context(tc.tile_pool(name="dram", bufs=2, space="DRAM"))

    idf = pc.tile([128, 128], DT_F32)
    make_identity(nc, idf)
    idb = pc.tile([128, 128], DT_BF)
    nc.vector.tensor_copy(out=idb, in_=idf)
    epst = pc.tile([128, 1], DT_F32)
    nc.vector.memset(epst, EPS)
    masks = pc.tile([128, KCH, R], DT_BF)
    nc.sync.dma_start(out=masks, in_=inp["masks"])
    biascol = pc.tile([128, KCH], DT_F32)
    nc.sync.dma_start(out=biascol, in_=inp["biascol"])
    csq = pc.tile([128, TT, 64], DT_BF)
    nc.sync.dma_start(out=csq, in_=inp["csq"])
    csk = pc.tile([128, TT, 64], DT_BF)
    nc.sync.dma_start(out=csk, in_=inp["csk"])

    x = pc.tile([128, TT, D], DT_F32)
    nc.sync.dma_start(out=x, in_=inp["x0"])

    def w_tile(name, ktiles, ncols, tag="wbig"):
        """DMA a [128, ktiles, ncols] bf16 weight (host pre-laid-out) to SBUF."""
        wt = wp.tile([128, ktiles, ncols], DT_BF, tag=tag)
        nc.sync.dma_start(out=wt, in_=name)
        return wt

    def layernorm(pool, tag):
        """rm layernorm of x -> bf16 [128, TT, D] (gain folded into weights)."""
        ln = pool.tile([128, TT, D], DT_BF, tag=tag)
        for t in range(TT):
            stats = pst.tile([128, 2, 6], DT_F32, tag="stats")
            for g2 in range(2):
                nc.vector.bn_stats(out=stats[:, g2, :], in_=x[:, t, g2 * 384:(g2 + 1) * 384])
            mv = pst.tile([128, 2], DT_F32, tag="mv")
            nc.vector.bn_aggr(out=mv, in_=stats)
            std = pst.tile([128, 1], DT_F32, tag="std")
            nc.scalar.activation(out=std, in_=mv[:, 1:2], func=AF.Sqrt, bias=epst)
            istd = pst.tile([128, 1], DT_F32, tag="istd")
            nc.vector.reciprocal(out=istd, in_=std)
            nc.vector.tensor_scalar(out=ln[:, t, :], in0=x[:, t, :],
                                    scalar1=mv[:, 0:1], scalar2=istd,
                                    op0=AL.subtract, op1=AL.mult)
        return ln

    def rm_to_fm(rm, pool, tag, src_f32=False):
        """Transpose [128, TT, D] rm -> [128, DTILES, R] fm bf16 via PE."""
        fm = pool.tile([128, DTILES, R], DT_BF, tag=tag)
        for dp in range(DTILES // 2):
            pt = ps512.tile([128, 2 * R], DT_F32 if src_f32 else DT_BF, tag="s512")
            for dd in range(2):
                d = 2 * dp + dd
                for t in range(TT):
                    nc.tensor.transpose(pt[:, dd * R + t * 128:dd * R + (t + 1) * 128],
                                        rm[:, t, d * 128:(d + 1) * 128],
                                        idf if src_f32 else idb)
            if dp % 2 == 0:
                nc.scalar.copy(out=fm[:, 2 * dp:2 * dp + 2, :], in_=pt)
            else:
                nc.vector.tensor_copy(out=fm[:, 2 * dp:2 * dp + 2, :], in_=pt)
        return fm

    def lateral(l, tag):
        with tc.tile_pool(name=f"l{l}lat{tag}", bufs=1) as pl:
            wproj = w_tile(inp[f"L{l}_proj{tag}"], DTILES, D)
            # two halves of the [2304, 768] extended gate weight
            wg0 = w_tile(inp[f"L{l}_gate{tag}"][:, 0:9, :], 9, D)
            wg1 = w_tile(inp[f"L{l}_gate{tag}"][:, 9:18, :], 9, D)
            lnl = layernorm(pl, "ln_rm")
            ln_fm = rm_to_fm(lnl, pl, "ln_fm")
            x_fm = rm_to_fm(x, pl, "x_fm", src_f32=True)
            # l_self = LN(x) @ proj, emitted feature-major
            lself = pl.tile([128, DTILES, R], DT_BF, tag="lself")
            for n in range(DTILES):
                pt = ps512.tile([128, R], DT_F32, tag="s512")
                for k in range(DTILES):
                    nc.tensor.matmul(pt, lhsT=wproj[:, k, n * 128:(n + 1) * 128],
                                     rhs=ln_fm[:, k, :],
                                     start=(k == 0), stop=(k == DTILES - 1))
                nc.scalar.copy(out=lself[:, n, :], in_=pt)
            ls = dram.tile([128, DTILES * R], DT_BF, tag="ls")
            nc.sync.dma_start(out=ls, in_=lself)
            lr = dram.tile([2, 128, DTILES * R], DT_BF, tag="lr")
            nc.gpsimd.collective_compute(
                "AllGather", AL.bypass, replica_groups=LAT_GROUPS,
                ins=[ls[:].opt()], outs=[lr[:].opt()])
            l01 = pl.tile([128, 2 * DTILES, R], DT_BF, tag="l01")
            for r2 in range(2):
                nc.sync.dma_start(
                    out=l01[:, r2 * DTILES:(r2 + 1) * DTILES, :],
                    in_=lr[r2].rearrange("p (k n) -> p k n", k=DTILES))
            # gate = sigmoid([x; recv0; recv1] @ [gx; A; B]); own-stream slot zeroed
            for t in range(TT):
                g = pl.tile([128, D], DT_BF, tag="g")
                for half in range(2):
                    pt = ps384.tile([128, 384], DT_F32, tag="p384")
                    for k in range(18):
                        if k < DTILES:
                            lhs = x_fm[:, k, t * 128:(t + 1) * 128]
                        else:
                            lhs = l01[:, k - DTILES, t * 128:(t + 1) * 128]
                        wg = wg0 if k < 9 else wg1
                        nc.tensor.matmul(pt, lhsT=lhs,
                                         rhs=wg[:, k % 9, half * 384:(half + 1) * 384],
                                         start=(k == 0), stop=(k == 17))
                    nc.scalar.activation(out=g[:, half * 384:(half + 1) * 384],
                                         in_=pt, func=AF.Sigmoid)
                nc.vector.tensor_tensor(out=x[:, t, :], in0=x[:, t, :], in1=g,
                                        op=AL.mult)

    def rope(psums, cs, dst, t):
        """Apply rotate-half rope to one token tile. psums = [psum heads 0-5,
        psum heads 6-11] each [128, 384] fp32; dst bf16 [128, TT, D]."""
        for ph in range(2):
            raw = pst.tile([128, 384], DT_BF, tag="rope_raw")
            nc.scalar.copy(out=raw, in_=psums[ph])
            rv = raw.rearrange("p (h two i) -> p h two i", h=6, two=2)
            dv = dst[:, t, ph * 384:(ph + 1) * 384].rearrange(
                "p (h two i) -> p h two i", h=6, two=2)
            c_ap = cs[:, t, 0:32]
            s_ap = cs[:, t, 32:64]
            cb = bass.AP(tensor=c_ap.tensor, offset=c_ap.offset,
                         ap=[c_ap.ap[0], [0, 6], c_ap.ap[1]])
            sb = bass.AP(tensor=s_ap.tensor, offset=s_ap.offset,
                         ap=[s_ap.ap[0], [0, 6], s_ap.ap[1]])
            t1 = pst.tile([128, 6, 32], DT_BF, tag="ropet1")
            t2 = pst.tile([128, 6, 32], DT_BF, tag="ropet2")
            q1, q2 = rv[:, :, 0, :], rv[:, :, 1, :]
            nc.vector.tensor_tensor(out=t1, in0=q1, in1=cb, op=AL.mult)
            nc.vector.tensor_tensor(out=t2, in0=q2, in1=sb, op=AL.mult)
            nc.vector.tensor_tensor(out=dv[:, :, 0, :], in0=t1, in1=t2, op=AL.subtract)
            nc.vector.tensor_tensor(out=t1, in0=q1, in1=sb, op=AL.mult)
            nc.vector.tensor_tensor(out=t2, in0=q2, in1=cb, op=AL.mult)
            nc.vector.tensor_tensor(out=dv[:, :, 1, :], in0=t1, in1=t2, op=AL.add)

    def attention(l):
        with tc.tile_pool(name=f"l{l}att", bufs=1) as pa:
            wlk = w_tile(inp[f"L{l}_lkT"], DTILES, NL, tag="wlk")
            wq = w_tile(inp[f"L{l}_wq"], DTILES, D)
            wk = w_tile(inp[f"L{l}_wk"], DTILES, D)
            wv = w_tile(inp[f"L{l}_wv"], DTILES, D)
            ln1 = layernorm(pa, "ln_rm")
            ln_fm = rm_to_fm(ln1, pa, "ln_fm")

            with tc.tile_pool(name=f"l{l}qkv", bufs=1) as pq:
                q_rm = pq.tile([128, TT, D], DT_BF, tag="q_rm")
                k_rm = pq.tile([128, TT, D], DT_BF, tag="k_rm")
                v_loc = pa.tile([128, TT, H * 65], DT_BF, tag="v_loc")
                for t in range(TT):
                    for w_, cs_, dst in ((wq, csq, q_rm), (wk, csk, k_rm)):
                        pts = []
                        for half in range(2):
                            pt = ps384.tile([128, 384], DT_F32, tag="p384")
                            for k in range(DTILES):
                                nc.tensor.matmul(
                                    pt, lhsT=ln_fm[:, k, t * 128:(t + 1) * 128],
                                    rhs=w_[:, k, half * 384:(half + 1) * 384],
                                    start=(k == 0), stop=(k == DTILES - 1))
                            pts.append(pt)
                        rope(pts, cs_, dst, t)
                    for half in range(2):
                        pt = ps384.tile([128, 384], DT_F32, tag="p384")
                        for k in range(DTILES):
                            nc.tensor.matmul(
                                pt, lhsT=ln_fm[:, k, t * 128:(t + 1) * 128],
                                rhs=wv[:, k, half * 384:(half + 1) * 384],
                                start=(k == 0), stop=(k == DTILES - 1))
                        # scatter the 6 heads of this half into v_aug layout
                        pv = pt.rearrange("p (h d) -> p h d", h=6)
                        vv = v_loc[:, t, half * 6 * 65:(half + 1) * 6 * 65]
                        vv = vv.rearrange("p (h d) -> p h d", h=6)
                        nc.vector.tensor_copy(out=vv[:, :, 0:64], in_=pv)
                    ones_cols = v_loc[:, t, :].rearrange("p (h d) -> p h d", h=H)
                    nc.vector.memset(ones_cols[:, :, 64:65], 1.0)
                q_fm = pa.tile([128, DTILES, R], DT_BF, tag="q_fm")
                k_fm = pa.tile([128, DTILES, R], DT_BF, tag="k_fm")
                for d in range(DTILES):
                    for src, dstt in ((q_rm, q_fm), (k_rm, k_fm)):
                        ptt = ps512.tile([128, R], DT_BF, tag="s512")
                        for t in range(TT):
                            nc.tensor.transpose(ptt[:, t * 128:(t + 1) * 128],
                                                src[:, t, d * 128:(d + 1) * 128], idb)
                        nc.scalar.copy(out=dstt[:, d, :], in_=ptt)

            # k/v exchange within token-half pair (global key order = rank order)
            KB = DTILES * R          # 3072 bf16 per partition for k
            VB = TT * H * 65         # 3120 for v
            kvs = dram.tile([128, KB + VB], DT_BF, tag="kvs")
            nc.sync.dma_start(out=kvs[:, 0:KB].rearrange("p (k n) -> p k n", k=DTILES),
                              in_=k_fm)
            nc.sync.dma_start(out=kvs[:, KB:].rearrange("p (t n) -> p t n", t=TT),
                              in_=v_loc)
            kvr = dram.tile([2, 128, KB + VB], DT_BF, tag="kvr")
            nc.gpsimd.collective_compute(
                "AllGather", AL.bypass, replica_groups=KV_GROUPS,
                ins=[kvs[:].opt()], outs=[kvr[:].opt()])
            k_all = pa.tile([128, DTILES, T + NL], DT_BF, tag="k_all")
            v_all = pa.tile([128, KCH + 1, H * 65], DT_BF, tag="v_all")
            for r2 in range(2):
                nc.sync.dma_start(
                    out=k_all[:, :, r2 * R:(r2 + 1) * R],
                    in_=kvr[r2, :, 0:KB].rearrange("p (k n) -> p k n", k=DTILES))
                nc.sync.dma_start(
                    out=v_all[:, r2 * TT:(r2 + 1) * TT, :],
                    in_=kvr[r2, :, KB:].rearrange("p (t n) -> p t n", t=TT))
            nc.sync.dma_start(out=k_all[:, :, T:T + NL], in_=wlk)
            nc.vector.memset(v_all[0:NL, KCH, :], 0.0)
            vlat = v_all[0:NL, KCH, :].rearrange("p (h d) -> p h d", h=H)
            nc.vector.memset(vlat[:, :, 64:65], 1.0)

            att_rm = pa.tile([128, TT, D], DT_BF, tag="att_rm")
            for h in range(H):
                po = 64 * (h % 2)
                ft = h // 2
                exp_sb = pa.tile([128, KCH, R], DT_BF, tag="exp_sb", bufs=2)
                exp_lat = pa.tile([NL, R], DT_BF, tag="exp_lat", bufs=2)
                for cp in range(KCH // 2):
                    pt = ps512.tile([128, 2 * R], DT_F32, tag="s512")
                    for dd in range(2):
                        ch = 2 * cp + dd
                        nc.tensor.matmul(
                            pt[:, dd * R:(dd + 1) * R],
                            lhsT=k_all[po:po + 64, ft, ch * 128:(ch + 1) * 128],
                            rhs=q_fm[po:po + 64, ft, :], start=True, stop=True)
                    nc.scalar.activation(out=exp_sb[:, 2 * cp:2 * cp + 2, :], in_=pt,
                                         func=AF.Exp, bias=biascol[:, 2 * cp:2 * cp + 1])
                    ext2 = 256 if cp % 2 == 0 else 512
                    nc.vector.tensor_tensor(out=exp_sb[:, 2 * cp:2 * cp + 2, 0:ext2],
                                            in0=exp_sb[:, 2 * cp:2 * cp + 2, 0:ext2],
                                            in1=masks[:, 2 * cp:2 * cp + 2, 0:ext2],
                                            op=AL.mult)
                ptl = ps512.tile([NL, R], DT_F32, tag="s512")
                nc.tensor.matmul(ptl, lhsT=wlk[po:po + 64, ft, :],
                                 rhs=q_fm[po:po + 64, ft, :], start=True, stop=True)
                nc.scalar.activation(out=exp_lat, in_=ptl, func=AF.Exp)
                for t in range(TT):
                    pav = psav.tile([128, 65], DT_F32, tag="pav")
                    for ch in range(KCH):
                        nc.tensor.matmul(pav,
                                         lhsT=exp_sb[:, ch, t * 128:(t + 1) * 128],
                                         rhs=v_all[:, ch, h * 65:(h + 1) * 65],
                                         start=(ch == 0), stop=False)
                    nc.tensor.matmul(pav, lhsT=exp_lat[:, t * 128:(t + 1) * 128],
                                     rhs=v_all[0:NL, KCH, h * 65:(h + 1) * 65],
                                     start=False, stop=True)
                    rec = pst.tile([128, 1], DT_F32, tag="rec")
                    nc.vector.reciprocal(out=rec, in_=pav[:, 64:65])
                    nc.vector.tensor_scalar(out=att_rm[:, t, h * 64:(h + 1) * 64],
                                            in0=pav[:, 0:64], scalar1=rec,
                                            scalar2=None, op0=AL.mult)

            att_fm = rm_to_fm(att_rm, pa, "att_fm")
            wo = w_tile(inp[f"L{l}_wo"], DTILES, D)
            for t in range(TT):
                for half in range(2):
                    pt = ps384.tile([128, 384], DT_F32, tag="p384")
                    for k in range(DTILES):
                        nc.tensor.matmul(pt, lhsT=att_fm[:, k, t * 128:(t + 1) * 128],
                                         rhs=wo[:, k, half * 384:(half + 1) * 384],
                                         start=(k == 0), stop=(k == DTILES - 1))
                    nc.vector.tensor_tensor(out=x[:, t, half * 384:(half + 1) * 384],
                                            in0=pt,
                                            in1=x[:, t, half * 384:(half + 1) * 384],
                                            op=AL.add)

    def ffn(l):
        with tc.tile_pool(name=f"l{l}ffn", bufs=1) as pf:
            ln2 = layernorm(pf, "ln_rm")
            ln_fm = rm_to_fm(ln2, pf, "ln_fm")
            mid = pf.tile([128, 24, R], DT_BF, tag="mid")
            for half in range(2):
                w1h = w_tile(inp[f"L{l}_w1"][:, :, half * 1536:(half + 1) * 1536], DTILES, 1536)
                for mp in range(6):
                    pt = ps512.tile([128, 2 * R], DT_F32, tag="s512")
                    for dd in range(2):
                        ml = 2 * mp + dd
                        for k in range(DTILES):
                            nc.tensor.matmul(pt[:, dd * R:(dd + 1) * R],
                                             lhsT=w1h[:, k, ml * 128:(ml + 1) * 128],
                                             rhs=ln_fm[:, k, :],
                                             start=(k == 0), stop=(k == DTILES - 1))
                    nc.scalar.activation(out=mid[:, half * 12 + 2 * mp:half * 12 + 2 * mp + 2, :],
                                         in_=pt, func=AF.Gelu)
            w2h0 = w_tile(inp[f"L{l}_w2"][:, 0:12, :], 12, D)
            w2h1 = w_tile(inp[f"L{l}_w2"][:, 12:24, :], 12, D)
            for t in range(TT):
                for half in range(2):
                    pt = ps384.tile([128, 384], DT_F32, tag="p384")
                    for m in range(24):
                        w2h = w2h0 if m < 12 else w2h1
                        nc.tensor.matmul(pt, lhsT=mid[:, m, t * 128:(t + 1) * 128],
                                         rhs=w2h[:, m % 12, half * 384:(half + 1) * 384],
                                         start=(m == 0), stop=(m == 23))
                    nc.vector.tensor_tensor(out=x[:, t, half * 384:(half + 1) * 384],
                                            in0=pt,
                                            in1=x[:, t, half * 384:(half + 1) * 384],
                                            op=AL.add)

    for l in range(nlayer):
        lateral(l, "pre")
        attention(l)
        lateral(l, "post")
        ffn(l)

    if out_dbg is not None:
        nc.sync.dma_start(out=out_dbg.rearrange("(t p) d -> p t d", p=128), in_=x)

    # final layernorm + exchange + vocab head
    with tc.tile_pool(name="head", bufs=1) as ph:
        lnf = layernorm(ph, "ln_rm")
        lnf_fm = rm_to_fm(lnf, ph, "ln_fm")
        fs = dram.tile([128, DTILES * R], DT_BF, tag="fs")
        nc.sync.dma_start(out=fs, in_=lnf_fm)
        fr = dram.tile([2, 128, DTILES * R], DT_BF, tag="fr")
        nc.gpsimd.collective_compute(
            "AllGather", AL.bypass, replica_groups=KV_GROUPS,
            ins=[fs[:].opt()], outs=[fr[:].opt()])
        lnf_all = ph.tile([128, DTILES, 2 * R], DT_BF, tag="lnf_all")
        for r2 in range(2):
            nc.sync.dma_start(
                out=lnf_all[:, :, r2 * R:(r2 + 1) * R],
                in_=fr[r2].rearrange("p (k n) -> p k n", k=DTILES))
        NV = 1000
        for vc in range(VH // NV):
            whs = wp.tile([128, DTILES, NV], DT_BF, tag="wbig")
            nc.sync.dma_start(out=whs, in_=inp["wh"][:, :, vc * NV:(vc + 1) * NV])
            for t8 in range(2 * TT):
                pt = ps512.tile([128, 2, 512], DT_F32, tag="s512")
                for half in range(2):
                    for k in range(DTILES):
                        nc.tensor.matmul(
                            pt[:, half, 0:500],
                            lhsT=lnf_all[:, k, t8 * 128:(t8 + 1) * 128],
                            rhs=whs[:, k, half * 500:(half + 1) * 500],
                            start=(k == 0), stop=(k == DTILES - 1))
                ot = ph.tile([128, NV], DT_F32, tag="ot", bufs=3)
                otv = ot.rearrange("p (two n) -> p two n", two=2)
                nc.vector.tensor_copy(out=otv, in_=pt[:, :, 0:500])
                nc.sync.dma_start(
                    out=out_log[t8 * 128:(t8 + 1) * 128, vc * NV:(vc + 1) * NV],
                    in_=ot)
    es.close()


# ----------------------------------------------------------------- interface

_PROG = {}


def _get_prog(nlayer=NLAYER, dbg=False):
    key = (nlayer, dbg)
    if key not in _PROG:
        _PROG[key] = _build(nlayer, dbg)
    return _PROG[key]


LAST_RESULT = None


def kernel(idx, params, _nlayer=NLAYER, _dbg=False):
    global LAST_RESULT
    idx = np.asarray(idx)
    in_maps = _prep_inputs(idx, params)
    nc = _get_prog(_nlayer, _dbg)
    res = run_bass_kernel_spmd(nc, in_maps, core_ids=list(range(N_CORES)))
    LAST_RESULT = res
    lm = np.empty((B, T, V), dtype=F32)
    la = np.empty((B, T, V), dtype=F32)
    for c in range(N_CORES):
        b, s, h = c >> 2, (c >> 1) & 1, c & 1
        dst = lm if s == 0 else la
        dst[b, :, h * VH:(h + 1) * VH] = res.results[c]["logits"]
    if _dbg:
        dbg = [res.results[c]["dbgx"] for c in range(N_CORES)]
        return (lm, la), dbg
    return (lm, la)
